# revision 1
# baseline (speedup 1.0000x reference)
"""Trainium2 Bass kernel for a K=1 neighborhood-attention block.

Reference computation (per batch b, N=2048 positions, C=512 channels):
    Q  = x @ Wq^T + bq ;  K = x @ Wk^T + bk ;  V = x @ Wv^T + bv
    s[n]   = Q[n] . K[nbr[n]] + rel_bias[0,0]
    scores = one-hot-sparse [N, N]: row n has s[n] at column nbr[n], zeros else
    probs  = softmax(scores / sqrt(C))
    out    = probs @ V[nbr] ;  y = out @ Wo^T + bo

Because each score row is all-zeros except one entry, softmax collapses:
    t[n]   = s[n] / sqrt(C); D = e^{t[n]} + (N-1)
    out[n] = (sum_m V[nbr[m]] + (e^{t[n]}-1) * V[nbr[nbr[n]]]) / D
With weight folding A = Wq^T Wk, B = Wv^T Wo^T, beta = Wo bv + bo:
    t[n] = (x[n] A xg[n]^T + x[n].u + xg[n].v + bq.bk + rb00)/sqrt(C)
    y[n] = w0[n] * S + w1[n] * P2[n]
      w0 = 1/(e^t + N-1), w1 = 1 - N*w0
      P2[n] = xg2[n] @ B + beta        (xg = x[nbr], xg2 = x[nbr[nbr]])
      S     = sxg @ B + N*beta         (sxg = sum_n xg[n])
Device work per core (1 batch): two [2048,512]x[512,512] matmuls (fp8
DoubleRow by default), a fused rowwise dot + exp, and a small vector
epilogue. Data-parallel over batch: 8 batches over 8 cores, weights
replicated host-side.
"""

import math
import os

import numpy as np

# Recover wedged NeuronCores from a previous crashed run at NRT init.
os.environ.setdefault("NEURON_RT_RESET_CORES", "1")

B, N, C = 8, 2048, 512
P = 128
NT = N // P          # 16 n-tiles
KC = C // P          # 4 contraction chunks
FD = 512             # matmul moving free dim / psum bank
GT = 2               # n-tiles per pipeline group
INV_SQRT_C = 1.0 / math.sqrt(C)

# main-matmul dtype: float8e4 (DoubleRow, fastest), bfloat16, float32r, float32
MM_DT = os.environ.get("NAB_MM_DT", "float8e4")

_TRACE = {"enabled": False, "trace_cores": None, "last": None}
_CACHE = {}


def _np_dt(name):
    import ml_dtypes

    return {
        "bfloat16": ml_dtypes.bfloat16,
        "float8e4": ml_dtypes.float8_e4m3,
    }.get(name, np.float32)


def _aux_name(mm_dt_str):
    return "float32" if mm_dt_str in ("float32", "float32r") else "bfloat16"


def _build_program(mm_dt_str, has_beta, has_sbias):
    import concourse.tile as tile
    from concourse import bacc, mybir
    from concourse.bass import ts

    mm_dt = getattr(mybir.dt, mm_dt_str)
    ax_dt = getattr(mybir.dt, _aux_name(mm_dt_str))
    f32 = mybir.dt.float32
    dr = mm_dt_str == "float8e4" and os.environ.get("NAB_DR", "1") == "1"
    kstep = 2 if dr else 1
    pmode = mybir.MatmulPerfMode.DoubleRow if dr else None

    nc = bacc.Bacc("TRN2", target_bir_lowering=False, debug=False)

    # ---- DRAM I/O (per core) ----
    xt_d = nc.dram_tensor("xt", [C, N], mm_dt, kind="ExternalInput")       # x^T
    xg2t_d = nc.dram_tensor("xg2t", [C, N], mm_dt, kind="ExternalInput")   # xg2^T
    xg8 = dr and os.environ.get("NAB_XG8", "0") == "1"
    xg_dt = mm_dt if xg8 else ax_dt
    xg_d = nc.dram_tensor("xg", [N, C], xg_dt, kind="ExternalInput")       # xg
    a_d = nc.dram_tensor("a", [C, C], mm_dt, kind="ExternalInput")         # A
    bm_d = nc.dram_tensor("bm", [C, C], mm_dt, kind="ExternalInput")       # B
    ones_d = nc.dram_tensor("ones1", [1, P], ax_dt, kind="ExternalInput")
    # rowc = beta - S''/N (accumulated into every P2 psum via a K=1 matmul);
    # s2bsrc = S''/N (broadcast across partitions for the final add)
    rowc_d = nc.dram_tensor("rowc", [1, FD], ax_dt, kind="ExternalInput")
    s2bsrc_d = nc.dram_tensor("s2bsrc", [1, FD], ax_dt, kind="ExternalInput")
    if has_sbias:
        sbias_d = nc.dram_tensor("sbias", [P, NT], f32, kind="ExternalInput")
    y_d = nc.dram_tensor("y", [N, C], ax_dt, kind="ExternalOutput")

    with tile.TileContext(nc) as tc:
        with (
            tc.tile_pool(name="singles", bufs=1) as singles,
            tc.tile_pool(name="scratch", bufs=3) as scratch,
            tc.tile_pool(name="xa_psum", bufs=3, space="PSUM") as xa_pool,
            tc.tile_pool(name="p2_psum", bufs=4, space="PSUM") as p2_pool,
        ):
            # ---- persistent SBUF ----
            xt_sb = singles.tile([P, KC, N], mm_dt)
            xg2t_sb = singles.tile([P, KC, N], mm_dt)
            xg_sb = singles.tile([P, NT, C], xg_dt)
            a_sb = singles.tile([P, KC, C], mm_dt)
            bm_sb = singles.tile([P, KC, C], mm_dt)
            ones_sb = singles.tile([1, P], ax_dt)
            rowc_sb = singles.tile([1, FD], ax_dt)
            s2b_sb = singles.tile([P, FD], ax_dt)
            s_all = singles.tile([P, NT], f32)
            e_all = singles.tile([P, NT], f32)
            w0_all = singles.tile([P, NT], f32)
            w1_all = singles.tile([P, NT], f32)

            # ---- constant / weight loads (ACT-side HWDGE queue) ----
            import concourse.bass as bass

            nc.scalar.dma_start(a_sb[:], a_d.ap().rearrange("(kc p) c -> p kc c", p=P))
            nc.scalar.dma_start(bm_sb[:], bm_d.ap().rearrange("(kc p) c -> p kc c", p=P))
            nc.scalar.dma_start(ones_sb[:], ones_d[:])
            nc.scalar.dma_start(rowc_sb[:], rowc_d[:])
            s2bsrc_ap = s2bsrc_d.ap()
            nc.gpsimd.dma_start(
                s2b_sb[:],
                bass.AP(
                    tensor=s2bsrc_ap.tensor,
                    offset=s2bsrc_ap.offset,
                    ap=[[0, P]] + list(s2bsrc_ap.ap)[1:],
                ),
            )

            # preload the ACT exp table while DMAs stream (one-time ~2.7us)
            warm = scratch.tile([1, 2], f32, tag="warm")
            nc.vector.memset(warm[:], 0.0)
            nc.scalar.activation(
                out=warm[:], in_=warm[:], func=mybir.ActivationFunctionType.Exp
            )
            if has_sbias:
                sbias_sb = singles.tile([P, NT], f32)
                nc.sync.dma_start(sbias_sb[:], sbias_d[:])

            xt_ap = xt_d.ap().rearrange("(kc p) n -> p kc n", p=P)
            xg2t_ap = xg2t_d.ap().rearrange("(kc p) n -> p kc n", p=P)
            xg_ap = xg_d.ap().rearrange("(nt p) c -> p nt c", p=P)
            y_ap = y_d.ap().rearrange("(nt p) c -> p nt c", p=P)

            # issue all big input DMAs up front, split across HWDGE queues;
            # the first xt/xg2t slivers are small so the PE starts early
            nc.sync.dma_start(xt_sb[:, :, 0:128], xt_ap[:, :, 0:128])
            nc.scalar.dma_start(xg2t_sb[:, :, 0:128], xg2t_ap[:, :, 0:128])
            nc.sync.dma_start(xt_sb[:, :, 128:512], xt_ap[:, :, 128:512])
            nc.scalar.dma_start(xg2t_sb[:, :, 128:512], xg2t_ap[:, :, 128:512])
            nc.sync.dma_start(xg_sb[:, 0:4, :], xg_ap[:, 0:4, :])
            for g in range(1, 4):
                nsl = slice(g * 512, (g + 1) * 512)
                nc.sync.dma_start(xt_sb[:, :, nsl], xt_ap[:, :, nsl])
                nc.scalar.dma_start(xg2t_sb[:, :, nsl], xg2t_ap[:, :, nsl])
                nc.sync.dma_start(
                    xg_sb[:, 4 * g : 4 * g + 4, :], xg_ap[:, 4 * g : 4 * g + 4, :]
                )

            # ---- pipelined per-group compute + softmax + epilogue ----
            for g in range(NT // GT):
                t0, t1 = GT * g, GT * g + GT
                p2_psums = {}
                for ti in range(t0, t1):
                    # XA = (x @ A) for this n-tile; then s = rowdot(XA, xg)
                    xa_psum = xa_pool.tile([P, FD], f32, tag="xa")
                    for kc in range(0, KC, kstep):
                        nc.tensor.matmul(
                            xa_psum[:],
                            xt_sb[:, kc : kc + kstep, ts(ti, P)],
                            a_sb[:, kc : kc + kstep, :],
                            start=(kc == 0),
                            stop=(kc + kstep == KC),
                            perf_mode=pmode,
                        )
                    prod = scratch.tile([P, FD], f32, tag="prod")
                    nc.vector.tensor_tensor(
                        prod[:], xa_psum[:], xg_sb[:, ti, :], mybir.AluOpType.mult
                    )
                    psink = scratch.tile([P, FD], ax_dt, tag="psink")
                    nc.scalar.activation(
                        out=psink[:],
                        in_=prod[:],
                        func=mybir.ActivationFunctionType.Copy,
                        accum_out=s_all[:, ti : ti + 1],
                    )
                    # P2 = xg2 @ B (+ beta); kept in PSUM until the epilogue
                    p2_psum = p2_pool.tile([P, FD], f32, tag="p2")
                    p2_psums[ti] = p2_psum
                    for kc in range(0, KC, kstep):
                        nc.tensor.matmul(
                            p2_psum[:],
                            xg2t_sb[:, kc : kc + kstep, ts(ti, P)],
                            bm_sb[:, kc : kc + kstep, :],
                            start=(kc == 0),
                            stop=(kc + kstep == KC and not has_beta),
                            perf_mode=pmode,
                        )
                    if has_beta:
                        nc.tensor.matmul(
                            p2_psum[:], ones_sb[:], rowc_sb[:], start=False, stop=True
                        )

                # softmax weights for this group: e = exp(t/sqrt(C));
                # w0 = 1/(e+N-1); w1 = 1 - N*w0
                gs = slice(t0, t1)
                if has_sbias:
                    nc.vector.tensor_tensor(
                        s_all[:, gs], s_all[:, gs], sbias_sb[:, gs], mybir.AluOpType.add
                    )
                nc.scalar.activation(
                    out=e_all[:, gs],
                    in_=s_all[:, gs],
                    func=mybir.ActivationFunctionType.Exp,
                    scale=INV_SQRT_C,
                )
                nc.vector.tensor_scalar_add(w1_all[:, gs], e_all[:, gs], float(N - 1))
                nc.vector.reciprocal(w0_all[:, gs], w1_all[:, gs])
                nc.vector.tensor_scalar(
                    out=w1_all[:, gs],
                    in0=w0_all[:, gs],
                    scalar1=float(-N),
                    scalar2=1.0,
                    op0=mybir.AluOpType.mult,
                    op1=mybir.AluOpType.add,
                )

                # epilogue: y[n] = w1[n]*H[n] + S''/N  (H = P2 - S''/N in PSUM)
                e1g = scratch.tile([P, GT, FD], ax_dt, tag="e1g")
                o_grp = scratch.tile([P, GT, FD], ax_dt, tag="ogrp")
                for ti in range(t0, t1):
                    if ti % 2 == 0:
                        nc.scalar.activation(
                            out=e1g[:, ti - t0, :],
                            in_=p2_psums[ti][:],
                            func=mybir.ActivationFunctionType.Copy,
                            scale=w1_all[:, ti : ti + 1],
                        )
                    else:
                        nc.vector.tensor_scalar_mul(
                            e1g[:, ti - t0, :], p2_psums[ti][:], w1_all[:, ti : ti + 1]
                        )
                e2g = scratch.tile([P, GT, FD], ax_dt, tag="e2g")
                for ti in range(t0, t1):
                    nc.vector.tensor_scalar_mul(
                        e2g[:, ti - t0, :], s2b_sb[:], w0_all[:, ti : ti + 1]
                    )
                nc.vector.tensor_tensor(
                    o_grp[:], e1g[:], e2g[:], mybir.AluOpType.add
                )
                nc.sync.dma_start(y_ap[:, t0:t1, :], o_grp[:])

    nc.compile()
    return nc


def kernel(x, neighbors, Wq, bq, Wk, bk, Wv, bv, rel_bias, Wo, bo):
    from concourse.bass_utils import run_bass_kernel_spmd

    x = np.asarray(x, dtype=np.float32)
    Wq = np.asarray(Wq, dtype=np.float32)
    Wk = np.asarray(Wk, dtype=np.float32)
    Wv = np.asarray(Wv, dtype=np.float32)
    Wo = np.asarray(Wo, dtype=np.float32)
    bq = np.asarray(bq, dtype=np.float32)
    bk = np.asarray(bk, dtype=np.float32)
    bv = np.asarray(bv, dtype=np.float32)
    bo = np.asarray(bo, dtype=np.float32)
    rel_bias = np.asarray(rel_bias, dtype=np.float32)
    nbr = np.asarray(neighbors).reshape(N, -1)[:, 0].astype(np.int64)
    nbr2 = nbr[nbr]

    mm_np = _np_dt(MM_DT)
    ax_np = _np_dt(_aux_name(MM_DT))

    # host-side weight folding (tiny)
    A = (Wq.T @ Wk).astype(np.float32)            # [C, C]
    Bm = (Wv.T @ Wo.T).astype(np.float32)         # [C, C]
    beta = (Wo @ bv + bo).astype(np.float32)      # [C]
    u = (Wq.T @ bk).astype(np.float32)
    v = (Wk.T @ bq).astype(np.float32)
    const = float(bq @ bk) + float(rel_bias[0, 0])

    xg = x[:, nbr, :]                             # [B, N, C]
    xg2 = x[:, nbr2, :]
    sxg = xg.sum(axis=1)                          # [B, C]
    # raw (pre-1/sqrt(C)) additive score bias; the scale is applied inside exp
    sbias = x @ u + xg @ v + const                # [B, N]

    S2 = (sxg @ Bm + float(N) * beta) / float(N)   # [B, C] = S''/N per batch

    has_beta = bool(np.any(beta != 0.0))
    has_sbias = bool(np.any(sbias != 0.0))

    key = (MM_DT, has_beta, has_sbias)
    if key not in _CACHE:
        _CACHE[key] = _build_program(*key)
    nc = _CACHE[key]

    ones1 = np.ones((1, P), dtype=ax_np)
    in_maps = []
    for b in range(B):
        m = {
            "xt": np.ascontiguousarray(x[b].T).astype(mm_np),
            "xg2t": np.ascontiguousarray(xg2[b].T).astype(mm_np),
            "xg": np.ascontiguousarray(xg[b]).astype(
                mm_np
                if (MM_DT == "float8e4" and os.environ.get("NAB_XG8", "0") == "1")
                else ax_np
            ),
            "a": A.astype(mm_np),
            "bm": Bm.astype(mm_np),
            "ones1": ones1,
            "rowc": beta[None, :].astype(ax_np),
            "s2bsrc": (float(N) * S2[b])[None, :].astype(ax_np),
        }
        if has_sbias:
            m["sbias"] = np.ascontiguousarray(sbias[b].reshape(NT, P).T).astype(
                np.float32
            )
        in_maps.append(m)

    res = run_bass_kernel_spmd(
        nc,
        in_maps,
        core_ids=list(range(B)),
        trace=_TRACE["enabled"],
        trace_cores=_TRACE["trace_cores"],
    )
    _TRACE["last"] = res
    y = np.stack([r["y"] for r in res.results], axis=0)
    return y.astype(np.float32)



# revision 3
# speedup vs baseline: 1.0684x; 1.0684x over previous
"""Trainium2 Bass kernel for a K=1 neighborhood-attention block.

Reference computation (per batch b, N=2048 positions, C=512 channels):
    Q  = x @ Wq^T + bq ;  K = x @ Wk^T + bk ;  V = x @ Wv^T + bv
    s[n]   = Q[n] . K[nbr[n]] + rel_bias[0,0]
    scores = one-hot-sparse [N, N]: row n has s[n] at column nbr[n], zeros else
    probs  = softmax(scores / sqrt(C))
    out    = probs @ V[nbr] ;  y = out @ Wo^T + bo

Because each score row is all-zeros except one entry, softmax collapses.
With weight folding A = Wq^T Wk, B = Wv^T Wo^T, beta = Wo bv + bo, and
S'' = sxg @ B + N*beta (sxg = sum_n x[nbr[n]]):
    t[n] = (x[n] A xg[n]^T + sbias[n]) / sqrt(C)
    w0 = 1/(e^t + N-1);  w1 = 1 - N*w0
    y[n] = w1[n] * (xg2[n] @ B + beta) + w0[n] * S''
         = w1[n] * H[n] + S''/N,   H = (xg2[n] - sxg/N) @ B
(the beta terms cancel inside H). The host ships xg2' = xg2 - sxg/N, so
the device computes H directly with one matmul and the epilogue is a
single fused scalar_tensor_tensor per tile: y = H*w1 + (S''/N).

Device work per core (1 batch): two [2048,512]x[512,512] fp8-DoubleRow
matmuls; per 128-row tile one fused rowdot (DVE scalar_tensor_tensor
with accum_out), one PSUM->SBUF evac (ACT), one fused epilogue (DVE);
per 4-tile group a tiny exp/reciprocal chain. Data-parallel over batch:
8 batches over 8 cores, weights replicated host-side.
"""

import math
import os

import numpy as np

# Recover wedged NeuronCores from a previous crashed run at NRT init.
os.environ.setdefault("NEURON_RT_RESET_CORES", "1")

B, N, C = 8, 2048, 512
P = 128
NT = N // P          # 16 n-tiles
KC = C // P          # 4 contraction chunks
FD = 512             # matmul moving free dim / psum bank
GT = 4               # n-tiles per softmax/epilogue group
INV_SQRT_C = 1.0 / math.sqrt(C)

# main-matmul dtype: float8e4 (DoubleRow, fastest), bfloat16, float32r, float32
MM_DT = os.environ.get("NAB_MM_DT", "float8e4")

_TRACE = {"enabled": False, "trace_cores": None, "last": None}
_CACHE = {}


def _np_dt(name):
    import ml_dtypes

    return {
        "bfloat16": ml_dtypes.bfloat16,
        "float8e4": ml_dtypes.float8_e4m3,
    }.get(name, np.float32)


def _aux_name(mm_dt_str):
    return "float32" if mm_dt_str in ("float32", "float32r") else "bfloat16"


def _build_program(mm_dt_str, has_sbias):
    import concourse.tile as tile
    from concourse import bacc, mybir
    from concourse.bass import ts

    mm_dt = getattr(mybir.dt, mm_dt_str)
    ax_dt = getattr(mybir.dt, _aux_name(mm_dt_str))
    f32 = mybir.dt.float32
    dr = mm_dt_str == "float8e4" and os.environ.get("NAB_DR", "1") == "1"
    kstep = 2 if dr else 1
    pmode = mybir.MatmulPerfMode.DoubleRow if dr else None

    nc = bacc.Bacc("TRN2", target_bir_lowering=False, debug=False)

    # ---- DRAM I/O (per core) ----
    xt_d = nc.dram_tensor("xt", [C, N], mm_dt, kind="ExternalInput")       # x^T
    xg2t_d = nc.dram_tensor("xg2t", [C, N], mm_dt, kind="ExternalInput")   # xg2'^T
    xg_d = nc.dram_tensor("xg", [N, C], ax_dt, kind="ExternalInput")       # xg
    a_d = nc.dram_tensor("a", [C, C], mm_dt, kind="ExternalInput")         # A
    bm_d = nc.dram_tensor("bm", [C, C], mm_dt, kind="ExternalInput")       # B
    # s2bsrc = S''/N (broadcast across partitions; added in the epilogue)
    s2bsrc_d = nc.dram_tensor("s2bsrc", [1, FD], ax_dt, kind="ExternalInput")
    if has_sbias:
        sbias_d = nc.dram_tensor("sbias", [P, NT], f32, kind="ExternalInput")
    y_d = nc.dram_tensor("y", [N, C], ax_dt, kind="ExternalOutput")

    with tile.TileContext(nc) as tc:
        with (
            tc.tile_pool(name="singles", bufs=1) as singles,
            tc.tile_pool(name="scratch", bufs=2) as scratch,
            tc.tile_pool(name="ogrp", bufs=2) as ogrp_pool,
            tc.tile_pool(name="xa_psum", bufs=4, space="PSUM") as xa_pool,
            tc.tile_pool(name="p2_psum", bufs=4, space="PSUM") as p2_pool,
        ):
            # ---- persistent SBUF ----
            xt_sb = singles.tile([P, KC, N], mm_dt)
            xg2t_sb = singles.tile([P, KC, N], mm_dt)
            xg_sb = singles.tile([P, NT, C], ax_dt)
            a_sb = singles.tile([P, KC, C], mm_dt)
            bm_sb = singles.tile([P, KC, C], mm_dt)
            s2b_sb = singles.tile([P, FD], ax_dt)
            h_sb = singles.tile([P, NT, FD], ax_dt)
            s_all = singles.tile([P, NT], f32)
            e_all = singles.tile([P, NT], f32)
            w0_all = singles.tile([P, NT], f32)
            w1_all = singles.tile([P, NT], f32)

            import concourse.bass as bass

            # broadcast S''/N across all 128 partitions (SWDGE, 0-stride AP)
            s2bsrc_ap = s2bsrc_d.ap()
            nc.gpsimd.dma_start(
                s2b_sb[:],
                bass.AP(
                    tensor=s2bsrc_ap.tensor,
                    offset=s2bsrc_ap.offset,
                    ap=[[0, P]] + list(s2bsrc_ap.ap)[1:],
                ),
            )

            # preload the ACT exp table while DMAs stream (one-time ~1.3us)
            warm = scratch.tile([1, 2], f32, tag="warm")
            nc.vector.memset(warm[:], 0.0)
            nc.scalar.activation(
                out=warm[:], in_=warm[:], func=mybir.ActivationFunctionType.Exp
            )
            if has_sbias:
                sbias_sb = singles.tile([P, NT], f32)
                nc.sync.dma_start(sbias_sb[:], sbias_d[:])

            xt_ap = xt_d.ap().rearrange("(kc p) n -> p kc n", p=P)
            xg2t_ap = xg2t_d.ap().rearrange("(kc p) n -> p kc n", p=P)
            xg_ap = xg_d.ap().rearrange("(nt p) c -> p nt c", p=P)
            y_ap = y_d.ap().rearrange("(nt p) c -> p nt c", p=P)

            # First-matmul deps first: xt sliver + A on separate queues, then
            # the partner weight, then stream the rest of the big inputs.
            nc.sync.dma_start(xt_sb[:, :, 0:128], xt_ap[:, :, 0:128])
            nc.scalar.dma_start(a_sb[:], a_d.ap().rearrange("(kc p) c -> p kc c", p=P))
            nc.sync.dma_start(xg2t_sb[:, :, 0:128], xg2t_ap[:, :, 0:128])
            nc.scalar.dma_start(bm_sb[:], bm_d.ap().rearrange("(kc p) c -> p kc c", p=P))
            nc.sync.dma_start(xt_sb[:, :, 128:1024], xt_ap[:, :, 128:1024])
            nc.scalar.dma_start(xg_sb[:, 0:4, :], xg_ap[:, 0:4, :])
            nc.sync.dma_start(xg2t_sb[:, :, 128:1024], xg2t_ap[:, :, 128:1024])
            nc.sync.dma_start(xt_sb[:, :, 1024:2048], xt_ap[:, :, 1024:2048])
            nc.scalar.dma_start(xg_sb[:, 4:10, :], xg_ap[:, 4:10, :])
            nc.sync.dma_start(xg2t_sb[:, :, 1024:2048], xg2t_ap[:, :, 1024:2048])
            nc.scalar.dma_start(xg_sb[:, 10:16, :], xg_ap[:, 10:16, :])

            # ---- pipelined per-tile compute ----
            for g in range(NT // GT):
                t0, t1 = GT * g, GT * g + GT
                for ti in range(t0, t1):
                    # XA = (x @ A) for this n-tile
                    xa_psum = xa_pool.tile([P, FD], f32, tag="xa")
                    for kc in range(0, KC, kstep):
                        nc.tensor.matmul(
                            xa_psum[:],
                            xt_sb[:, kc : kc + kstep, ts(ti, P)],
                            a_sb[:, kc : kc + kstep, :],
                            start=(kc == 0),
                            stop=(kc + kstep == KC),
                            perf_mode=pmode,
                        )
                    # H = xg2' @ B for this n-tile
                    p2_psum = p2_pool.tile([P, FD], f32, tag="p2")
                    for kc in range(0, KC, kstep):
                        nc.tensor.matmul(
                            p2_psum[:],
                            xg2t_sb[:, kc : kc + kstep, ts(ti, P)],
                            bm_sb[:, kc : kc + kstep, :],
                            start=(kc == 0),
                            stop=(kc + kstep == KC),
                            perf_mode=pmode,
                        )
                    # fused rowdot: s[n] = sum_c XA[n,c]*xg[n,c] (one DVE op)
                    sink = scratch.tile([P, FD], ax_dt, tag="sink")
                    nc.vector.scalar_tensor_tensor(
                        out=sink[:],
                        in0=xa_psum[:],
                        scalar=1.0,
                        in1=xg_sb[:, ti, :],
                        op0=mybir.AluOpType.mult,
                        op1=mybir.AluOpType.mult,
                        accum_out=s_all[:, ti : ti + 1],
                    )
                    # evacuate H to SBUF (ACT; frees the PSUM bank quickly)
                    nc.scalar.activation(
                        out=h_sb[:, ti, :],
                        in_=p2_psum[:],
                        func=mybir.ActivationFunctionType.Copy,
                    )

                # softmax weights for this group: e = exp(t/sqrt(C));
                # w0 = 1/(e+N-1); w1 = 1 - N*w0
                gs = slice(t0, t1)
                if has_sbias:
                    nc.vector.tensor_tensor(
                        s_all[:, gs], s_all[:, gs], sbias_sb[:, gs], mybir.AluOpType.add
                    )
                nc.scalar.activation(
                    out=e_all[:, gs],
                    in_=s_all[:, gs],
                    func=mybir.ActivationFunctionType.Exp,
                    scale=INV_SQRT_C,
                )
                nc.vector.tensor_scalar_add(w1_all[:, gs], e_all[:, gs], float(N - 1))
                nc.vector.reciprocal(w0_all[:, gs], w1_all[:, gs])
                nc.vector.tensor_scalar(
                    out=w1_all[:, gs],
                    in0=w0_all[:, gs],
                    scalar1=float(-N),
                    scalar2=1.0,
                    op0=mybir.AluOpType.mult,
                    op1=mybir.AluOpType.add,
                )

                # epilogue: y = H*w1 + S''/N  (one DVE op per tile)
                o_grp = ogrp_pool.tile([P, GT, FD], ax_dt, tag="ogrp")
                for ti in range(t0, t1):
                    nc.vector.scalar_tensor_tensor(
                        out=o_grp[:, ti - t0, :],
                        in0=h_sb[:, ti, :],
                        scalar=w1_all[:, ti : ti + 1],
                        in1=s2b_sb[:],
                        op0=mybir.AluOpType.mult,
                        op1=mybir.AluOpType.add,
                    )
                nc.sync.dma_start(y_ap[:, t0:t1, :], o_grp[:])

    nc.compile()
    return nc


def kernel(x, neighbors, Wq, bq, Wk, bk, Wv, bv, rel_bias, Wo, bo):
    from concourse.bass_utils import run_bass_kernel_spmd

    x = np.asarray(x, dtype=np.float32)
    Wq = np.asarray(Wq, dtype=np.float32)
    Wk = np.asarray(Wk, dtype=np.float32)
    Wv = np.asarray(Wv, dtype=np.float32)
    Wo = np.asarray(Wo, dtype=np.float32)
    bq = np.asarray(bq, dtype=np.float32)
    bk = np.asarray(bk, dtype=np.float32)
    bv = np.asarray(bv, dtype=np.float32)
    bo = np.asarray(bo, dtype=np.float32)
    rel_bias = np.asarray(rel_bias, dtype=np.float32)
    nbr = np.asarray(neighbors).reshape(N, -1)[:, 0].astype(np.int64)
    nbr2 = nbr[nbr]

    mm_np = _np_dt(MM_DT)
    ax_np = _np_dt(_aux_name(MM_DT))

    # host-side weight folding (tiny)
    A = (Wq.T @ Wk).astype(np.float32)            # [C, C]
    Bm = (Wv.T @ Wo.T).astype(np.float32)         # [C, C]
    beta = (Wo @ bv + bo).astype(np.float32)      # [C]
    u = (Wq.T @ bk).astype(np.float32)
    v = (Wk.T @ bq).astype(np.float32)
    const = float(bq @ bk) + float(rel_bias[0, 0])

    xg = x[:, nbr, :]                             # [B, N, C]
    # xg2' = xg2 - mean of gathered rows; beta cancels inside H = xg2' @ B
    xg2p = x[:, nbr2, :] - xg.mean(axis=1, keepdims=True)
    # raw (pre-1/sqrt(C)) additive score bias; the scale is applied inside exp
    sbias = x @ u + xg @ v + const                # [B, N]

    S2 = (xg.sum(axis=1) @ Bm) / float(N) + beta  # [B, C] = S''/N per batch

    has_sbias = bool(np.any(sbias != 0.0))

    key = (MM_DT, has_sbias)
    if key not in _CACHE:
        _CACHE[key] = _build_program(*key)
    nc = _CACHE[key]

    in_maps = []
    for b in range(B):
        m = {
            "xt": np.ascontiguousarray(x[b].T).astype(mm_np),
            "xg2t": np.ascontiguousarray(xg2p[b].T).astype(mm_np),
            "xg": np.ascontiguousarray(xg[b]).astype(ax_np),
            "a": A.astype(mm_np),
            "bm": Bm.astype(mm_np),
            "s2bsrc": S2[b][None, :].astype(ax_np),
        }
        if has_sbias:
            m["sbias"] = np.ascontiguousarray(sbias[b].reshape(NT, P).T).astype(
                np.float32
            )
        in_maps.append(m)

    res = run_bass_kernel_spmd(
        nc,
        in_maps,
        core_ids=list(range(B)),
        trace=_TRACE["enabled"],
        trace_cores=_TRACE["trace_cores"],
    )
    _TRACE["last"] = res
    y = np.stack([r["y"] for r in res.results], axis=0)
    return y.astype(np.float32)


# revision 4
# speedup vs baseline: 1.0807x; 1.0115x over previous
"""Trainium2 Bass kernel for a K=1 neighborhood-attention block.

Reference computation (per batch b, N=2048 positions, C=512 channels):
    Q  = x @ Wq^T + bq ;  K = x @ Wk^T + bk ;  V = x @ Wv^T + bv
    s[n]   = Q[n] . K[nbr[n]] + rel_bias[0,0]
    scores = one-hot-sparse [N, N]: row n has s[n] at column nbr[n], zeros else
    probs  = softmax(scores / sqrt(C))
    out    = probs @ V[nbr] ;  y = out @ Wo^T + bo

Because each score row is all-zeros except one entry, softmax collapses.
With weight folding A = Wq^T Wk, B = Wv^T Wo^T, beta = Wo bv + bo, and
S'' = sxg @ B + N*beta (sxg = sum_n x[nbr[n]]):
    t[n] = (x[n] A xg[n]^T + sbias[n]) / sqrt(C)
    w0 = 1/(e^t + N-1);  w1 = 1 - N*w0
    y[n] = w1[n] * H[n] + S''/N,   H = (xg2[n] - sxg/N) @ B
(the beta terms cancel inside H). The host ships xg2' = xg2 - sxg/N, so
the device computes H directly with one matmul; the w1 multiply rides the
mandatory PSUM->SBUF evacuation (ACT activation scale), and the +S''/N
is a single bf16 tensor add per tile (GpSimd, last group on DVE).

All DRAM inputs/outputs are laid out host-side as [128 partitions,
contiguous-per-partition] so every DMA is 128 fat descriptors.

Device work per core (1 batch): two [2048,512]x[512,512] fp8-DoubleRow
matmuls; per 128-row tile one fused rowdot (DVE scalar_tensor_tensor
with accum_out), one scaled evac (ACT), one add (GpSimd/DVE); per
4-tile group a tiny exp/reciprocal chain. Data-parallel over batch:
8 batches over 8 cores, weights replicated host-side.
"""

import math
import os

import numpy as np

# Recover wedged NeuronCores from a previous crashed run at NRT init.
os.environ.setdefault("NEURON_RT_RESET_CORES", "1")

B, N, C = 8, 2048, 512
P = 128
NT = N // P          # 16 n-tiles
KC = C // P          # 4 contraction chunks
FD = 512             # matmul moving free dim / psum bank
GT = 4               # n-tiles per softmax/epilogue group
INV_SQRT_C = 1.0 / math.sqrt(C)

# main-matmul dtype: float8e4 (DoubleRow, fastest), bfloat16, float32r, float32
MM_DT = os.environ.get("NAB_MM_DT", "float8e4")

_TRACE = {"enabled": False, "trace_cores": None, "last": None}
_CACHE = {}


def _np_dt(name):
    import ml_dtypes

    return {
        "bfloat16": ml_dtypes.bfloat16,
        "float8e4": ml_dtypes.float8_e4m3,
    }.get(name, np.float32)


def _aux_name(mm_dt_str):
    return "float32" if mm_dt_str in ("float32", "float32r") else "bfloat16"


def _build_program(mm_dt_str, has_sbias):
    import concourse.tile as tile
    from concourse import bacc, mybir

    mm_dt = getattr(mybir.dt, mm_dt_str)
    ax_dt = getattr(mybir.dt, _aux_name(mm_dt_str))
    f32 = mybir.dt.float32
    dr = mm_dt_str == "float8e4" and os.environ.get("NAB_DR", "1") == "1"
    kstep = 2 if dr else 1
    pmode = mybir.MatmulPerfMode.DoubleRow if dr else None

    nc = bacc.Bacc("TRN2", target_bir_lowering=False, debug=False)

    # ---- DRAM I/O (per core); all pre-tiled host-side: partition dim first,
    # per-partition data contiguous ----
    xt_d = nc.dram_tensor("xt", [P, NT, KC, P], mm_dt, kind="ExternalInput")
    xg2t_d = nc.dram_tensor("xg2t", [P, NT, KC, P], mm_dt, kind="ExternalInput")
    xg_d = nc.dram_tensor("xg", [P, NT, C], ax_dt, kind="ExternalInput")
    a_d = nc.dram_tensor("a", [P, KC, C], mm_dt, kind="ExternalInput")
    bm_d = nc.dram_tensor("bm", [P, KC, C], mm_dt, kind="ExternalInput")
    # s2bsrc = S''/N (broadcast across partitions; added in the epilogue)
    s2bsrc_d = nc.dram_tensor("s2bsrc", [1, FD], ax_dt, kind="ExternalInput")
    if has_sbias:
        sbias_d = nc.dram_tensor("sbias", [P, NT], f32, kind="ExternalInput")
    y_d = nc.dram_tensor("y", [P, NT, C], ax_dt, kind="ExternalOutput")

    with tile.TileContext(nc) as tc:
        with (
            tc.tile_pool(name="singles", bufs=1) as singles,
            tc.tile_pool(name="scratch", bufs=2) as scratch,
            tc.tile_pool(name="ogrp", bufs=2) as ogrp_pool,
            tc.tile_pool(name="xa_psum", bufs=4, space="PSUM") as xa_pool,
            tc.tile_pool(name="p2_psum", bufs=4, space="PSUM") as p2_pool,
        ):
            # ---- persistent SBUF ----
            xt_sb = singles.tile([P, NT, KC, P], mm_dt)
            xg2t_sb = singles.tile([P, NT, KC, P], mm_dt)
            xg_sb = singles.tile([P, NT, C], ax_dt)
            a_sb = singles.tile([P, KC, C], mm_dt)
            bm_sb = singles.tile([P, KC, C], mm_dt)
            s2b_sb = singles.tile([P, FD], ax_dt)
            s_all = singles.tile([P, NT], f32)
            e_all = singles.tile([P, NT], f32)
            w0_all = singles.tile([P, NT], f32)
            w1_all = singles.tile([P, NT], f32)

            import concourse.bass as bass

            xt_ap = xt_d.ap()
            xg2t_ap = xg2t_d.ap()
            xg_ap = xg_d.ap()
            y_ap = y_d.ap()

            # First-matmul deps first: xt/xg2t slivers + weights, then the
            # bulk, interleaved across the two HWDGE queues.
            nc.sync.dma_start(xt_sb[:, 0:2], xt_ap[:, 0:2])
            nc.scalar.dma_start(a_sb[:], a_d.ap())
            nc.sync.dma_start(xg2t_sb[:, 0:2], xg2t_ap[:, 0:2])
            nc.scalar.dma_start(bm_sb[:], bm_d.ap())
            nc.sync.dma_start(xt_sb[:, 2:8], xt_ap[:, 2:8])
            nc.scalar.dma_start(xg_sb[:, 0:4], xg_ap[:, 0:4])
            nc.sync.dma_start(xg2t_sb[:, 2:8], xg2t_ap[:, 2:8])
            nc.sync.dma_start(xt_sb[:, 8:16], xt_ap[:, 8:16])
            nc.scalar.dma_start(xg_sb[:, 4:10], xg_ap[:, 4:10])
            nc.sync.dma_start(xg2t_sb[:, 8:16], xg2t_ap[:, 8:16])
            nc.scalar.dma_start(xg_sb[:, 10:16], xg_ap[:, 10:16])

            # broadcast S''/N across all 128 partitions (SWDGE, 0-stride AP)
            s2bsrc_ap = s2bsrc_d.ap()
            nc.gpsimd.dma_start(
                s2b_sb[:],
                bass.AP(
                    tensor=s2bsrc_ap.tensor,
                    offset=s2bsrc_ap.offset,
                    ap=[[0, P]] + list(s2bsrc_ap.ap)[1:],
                ),
            )
            if has_sbias:
                sbias_sb = singles.tile([P, NT], f32)
                nc.sync.dma_start(sbias_sb[:], sbias_d[:])

            # preload the ACT exp table (after the critical DMA issues)
            warm = scratch.tile([1, 2], f32, tag="warm")
            nc.vector.memset(warm[:], 0.0)
            nc.scalar.activation(
                out=warm[:], in_=warm[:], func=mybir.ActivationFunctionType.Exp
            )

            # ---- pipelined per-group compute ----
            for g in range(NT // GT):
                t0, t1 = GT * g, GT * g + GT
                gs = slice(t0, t1)
                xa_psums = {}
                # phase 1: XA matmuls + fused rowdot scores
                for ti in range(t0, t1):
                    xa_psum = xa_pool.tile([P, FD], f32, tag="xa")
                    xa_psums[ti] = xa_psum
                    for kc in range(0, KC, kstep):
                        nc.tensor.matmul(
                            xa_psum[:],
                            xt_sb[:, ti, kc : kc + kstep, :],
                            a_sb[:, kc : kc + kstep, :],
                            start=(kc == 0),
                            stop=(kc + kstep == KC),
                            perf_mode=pmode,
                        )
                    # fused rowdot: s[n] = sum_c XA[n,c]*xg[n,c] (one DVE op)
                    sink = scratch.tile([P, FD], ax_dt, tag="sink")
                    nc.vector.scalar_tensor_tensor(
                        out=sink[:],
                        in0=xa_psum[:],
                        scalar=1.0,
                        in1=xg_sb[:, ti, :],
                        op0=mybir.AluOpType.mult,
                        op1=mybir.AluOpType.mult,
                        accum_out=s_all[:, ti : ti + 1],
                    )

                # softmax weights: e = exp(t/sqrt(C)); w0 = 1/(e+N-1);
                # w1 = 1 - N*w0
                if has_sbias:
                    nc.vector.tensor_tensor(
                        s_all[:, gs], s_all[:, gs], sbias_sb[:, gs], mybir.AluOpType.add
                    )
                nc.scalar.activation(
                    out=e_all[:, gs],
                    in_=s_all[:, gs],
                    func=mybir.ActivationFunctionType.Exp,
                    scale=INV_SQRT_C,
                )
                nc.vector.tensor_scalar_add(w1_all[:, gs], e_all[:, gs], float(N - 1))
                nc.vector.reciprocal(w0_all[:, gs], w1_all[:, gs])
                nc.vector.tensor_scalar(
                    out=w1_all[:, gs],
                    in0=w0_all[:, gs],
                    scalar1=float(-N),
                    scalar2=1.0,
                    op0=mybir.AluOpType.mult,
                    op1=mybir.AluOpType.add,
                )

                # phase 2: H matmuls; evac rides the w1 scale (ACT), then
                # the +S''/N add (GpSimd; last group on DVE for a short tail)
                o_grp = ogrp_pool.tile([P, GT, FD], ax_dt, tag="ogrp")
                for ti in range(t0, t1):
                    p2_psum = p2_pool.tile([P, FD], f32, tag="p2")
                    for kc in range(0, KC, kstep):
                        nc.tensor.matmul(
                            p2_psum[:],
                            xg2t_sb[:, ti, kc : kc + kstep, :],
                            bm_sb[:, kc : kc + kstep, :],
                            start=(kc == 0),
                            stop=(kc + kstep == KC),
                            perf_mode=pmode,
                        )
                    nc.scalar.activation(
                        out=o_grp[:, ti - t0, :],
                        in_=p2_psum[:],
                        func=mybir.ActivationFunctionType.Copy,
                        scale=w1_all[:, ti : ti + 1],
                    )
                    eng = nc.vector if g == NT // GT - 1 else nc.gpsimd
                    eng.tensor_tensor(
                        o_grp[:, ti - t0, :],
                        o_grp[:, ti - t0, :],
                        s2b_sb[:],
                        mybir.AluOpType.add,
                    )
                nc.sync.dma_start(y_ap[:, gs], o_grp[:])

    nc.compile()
    return nc


def kernel(x, neighbors, Wq, bq, Wk, bk, Wv, bv, rel_bias, Wo, bo):
    from concourse.bass_utils import run_bass_kernel_spmd

    x = np.asarray(x, dtype=np.float32)
    Wq = np.asarray(Wq, dtype=np.float32)
    Wk = np.asarray(Wk, dtype=np.float32)
    Wv = np.asarray(Wv, dtype=np.float32)
    Wo = np.asarray(Wo, dtype=np.float32)
    bq = np.asarray(bq, dtype=np.float32)
    bk = np.asarray(bk, dtype=np.float32)
    bv = np.asarray(bv, dtype=np.float32)
    bo = np.asarray(bo, dtype=np.float32)
    rel_bias = np.asarray(rel_bias, dtype=np.float32)
    nbr = np.asarray(neighbors).reshape(N, -1)[:, 0].astype(np.int64)
    nbr2 = nbr[nbr]

    mm_np = _np_dt(MM_DT)
    ax_np = _np_dt(_aux_name(MM_DT))

    # host-side weight folding (tiny)
    A = (Wq.T @ Wk).astype(np.float32)            # [C, C]
    Bm = (Wv.T @ Wo.T).astype(np.float32)         # [C, C]
    beta = (Wo @ bv + bo).astype(np.float32)      # [C]
    u = (Wq.T @ bk).astype(np.float32)
    v = (Wk.T @ bq).astype(np.float32)
    const = float(bq @ bk) + float(rel_bias[0, 0])

    xg = x[:, nbr, :]                             # [B, N, C]
    # xg2' = xg2 - mean of gathered rows; beta cancels inside H = xg2' @ B
    xg2p = x[:, nbr2, :] - xg.mean(axis=1, keepdims=True)
    # raw (pre-1/sqrt(C)) additive score bias; the scale is applied inside exp
    sbias = x @ u + xg @ v + const                # [B, N]

    S2 = (xg.sum(axis=1) @ Bm) / float(N) + beta  # [B, C] = S''/N per batch

    has_sbias = bool(np.any(sbias != 0.0))

    key = (MM_DT, has_sbias)
    if key not in _CACHE:
        _CACHE[key] = _build_program(*key)
    nc = _CACHE[key]

    def tile_T(t):  # [N, C] -> [P, NT, KC, P] (x^T pre-tiled per partition)
        return np.ascontiguousarray(
            t.reshape(NT, P, KC, P).transpose(3, 0, 2, 1)
        )

    def tile_n(t):  # [N, C] -> [P, NT, C]
        return np.ascontiguousarray(t.reshape(NT, P, C).transpose(1, 0, 2))

    A_t = np.ascontiguousarray(A.reshape(KC, P, C).transpose(1, 0, 2)).astype(mm_np)
    Bm_t = np.ascontiguousarray(Bm.reshape(KC, P, C).transpose(1, 0, 2)).astype(mm_np)

    in_maps = []
    for b in range(B):
        m = {
            "xt": tile_T(x[b]).astype(mm_np),
            "xg2t": tile_T(xg2p[b]).astype(mm_np),
            "xg": tile_n(xg[b]).astype(ax_np),
            "a": A_t,
            "bm": Bm_t,
            "s2bsrc": S2[b][None, :].astype(ax_np),
        }
        if has_sbias:
            m["sbias"] = np.ascontiguousarray(sbias[b].reshape(NT, P).T).astype(
                np.float32
            )
        in_maps.append(m)

    res = run_bass_kernel_spmd(
        nc,
        in_maps,
        core_ids=list(range(B)),
        trace=_TRACE["enabled"],
        trace_cores=_TRACE["trace_cores"],
    )
    _TRACE["last"] = res
    # y comes back [P, NT, C]; n = nt*P + p
    y = np.stack(
        [r["y"].transpose(1, 0, 2).reshape(N, C) for r in res.results], axis=0
    )
    return y.astype(np.float32)
